# revision 7
# baseline (speedup 1.0000x reference)
"""Trainium2 Bass kernel for nn_DeltaSynapse.

I[b,o] = einsum('beo,dbe,deo,dbe->bo', Weff, Xd, delaymap, Wshort+1)
with Weff[b,e,o] = signs[e,o] * (W[e,o]*(1-frac[e,o]) + Wlong[b,e,o]*frac[e,o])

Identity: I[b,o] = sum_e H2[b,e,o] * Weff[b,e,o],
          H2[b,e,o] = sum_d G[d,b,e] * dm[d,e,o],  G = Xd*(Wshort+1).

Host computes Weff and the block-diagonal stationary gb
(gb[(d,j),(bb,j')] = G[d,b,e]*delta_{j,j'}); o-dim sharded 8 ways
(no=256/core). Two per-block orientations balance PE vs DVE/GpSimd:

A-blocks (gc < NA): H2 matmul per b-half (stationary gb, stream dm),
  Z = Hp*Weff on DVE/GpSimd, b-sum via eh matmuls into PSUM [16,(s,o)],
  folded once at the end (PE does the partition reduction).
B-blocks: transposed H2 (stationary dm o-chunk, stream gb) ->
  HpT[o,(h,m)]; Z on DVE/GpSimd; b-sum = DVE tensor_reduce over j'
  into per-group slots, one end reduce -> outT[o,(c,h,bb)] (DVE does
  the reduction, freeing the PE).
Both orientations read the same dm/gb tiles; Weff is shipped in the
per-block-type layout (same bytes). Host adds the two outputs.
"""

import os
import sys
import numpy as np

sys.path.insert(0, "/opt/trn_rl_repo")

import ml_dtypes

BF16 = ml_dtypes.bfloat16

# problem constants
D, B, N = 8, 16, 2048
NCORES = 8
NO = N // NCORES  # per-core o-slice width (256)
J = 16            # e's per group
NG = N // J       # e-groups (128)
HB = B // 2       # b per half (8)
C = 8             # groups per DMA block
NB = NG // C      # DMA blocks (16)
NA = 7            # A-orientation blocks (PE Zred); rest are B (DVE reduce)


def _consts():
    # eh[h, p=(bb,j'), b] = 1 iff b == h*HB+bb  (bb-major partitions)
    eh = np.zeros((2, 128, B), dtype=np.float32)
    for h in range(2):
        for bb in range(HB):
            eh[h, bb * J:(bb + 1) * J, h * HB + bb] = 1.0
    return eh


def host_prep(W, Wlong, Wshort, Xd, delaymap, STDP_frac, signs_pre, use_bf16=True):
    """Host-side prep: Weff fusion, block-diag gb, layout transforms, o-shard."""
    dt = BF16 if use_bf16 else np.float32
    W = np.asarray(W, np.float32)
    frac = np.asarray(STDP_frac, np.float32)
    signs = np.where(W > 0, np.sign(np.asarray(signs_pre, np.float32))[:, None],
                     np.float32(0.0))
    A = signs * W * (1.0 - frac)
    SF = signs * frac
    Weff = (A[None] + SF[None] * np.asarray(Wlong, np.float32))  # [B,N,N] f32
    G = (np.asarray(Xd, np.float32) *
         (np.asarray(Wshort, np.float32) + 1.0))  # [D,B,N]

    # dm_r[gc, p=(d,j), (s,o)] = dm[d, (gc*C+s)*J+j, c*NO+o]
    dmf = np.asarray(delaymap, np.float32)
    dm5 = dmf.reshape(D, NB, C, J, N).transpose(1, 0, 3, 2, 4)  # [NB,D,J,C,N]

    # A-blocks: wfA[gc, p=(bb,j'), (h,s,o)]
    wfA = Weff.reshape(2, HB, NB, C, J, N).transpose(2, 1, 4, 0, 3, 5)
    # [NB, HB, J, 2, C, N]
    # B-blocks: wfT[gc, p, (s,c2,h,bb,j')], o = core*NO + c2*128 + p
    wfT = Weff.reshape(2, HB, NB, C, J, NCORES, 2, 128)
    wfT = wfT.transpose(5, 2, 7, 3, 6, 0, 1, 4)  # [core, gc, p, s, c2, h, bb, j']

    # gb[gc, p=(d,j), (s,h,m=(bb,j'))] (core-independent)
    gbar = np.zeros((NB, D, J, C, 2, HB, J), np.float32)
    Gr = G.reshape(D, 2, HB, NB, C, J)  # [d,h,bb,gc,s,j]
    for j in range(J):
        gbar[:, :, j, :, :, :, j] = Gr[:, :, :, :, :, j].transpose(3, 0, 4, 1, 2)
    gb = np.ascontiguousarray(gbar.reshape(NB, 128, C * 2 * 128)).astype(dt)

    ins = []
    for c in range(NCORES):
        sl = slice(c * NO, (c + 1) * NO)
        wa = np.ascontiguousarray(
            wfA[:NA, :, :, :, :, sl]).reshape(NA, 128, 2 * C * NO).astype(dt)
        wb = np.ascontiguousarray(
            wfT[c, NA:]).reshape(NB - NA, 128, C * 2 * 2 * HB * J).astype(dt)
        ins.append({
            "dm": np.ascontiguousarray(
                dm5[:, :, :, :, sl].reshape(NB, 128, C * NO)).astype(dt),
            "wf": np.concatenate([wa, wb], axis=0),  # [NB, 128, 4096]
            "gb": gb,
        })
    return ins


def build_nc(use_bf16=True, n_cores=NCORES, no=NO, ng=NG):
    """Build the SPMD Bass program (same on all cores)."""
    import concourse.bass as bass
    import concourse.bacc as bacc
    import concourse.mybir as mybir
    import concourse.tile as tile
    from contextlib import ExitStack

    dt_big = mybir.dt.bfloat16 if use_bf16 else mybir.dt.float32
    f32 = mybir.dt.float32
    nb = ng // C
    nbgB = (nb - NA) * C  # number of B-orientation groups

    nc = bacc.Bacc("TRN2", target_bir_lowering=False, debug=False,
                   num_devices=n_cores)

    dm = nc.declare_dram_parameter("dm", [nb, 128, C * no], dt_big, isOutput=False).ap()
    wf = nc.declare_dram_parameter("wf", [nb, 128, 2 * C * no], dt_big, isOutput=False).ap()
    gb = nc.declare_dram_parameter("gb", [nb, 128, C * 2 * 128], dt_big, isOutput=False).ap()
    out = nc.declare_dram_parameter("out", [B, no], f32, isOutput=True).ap()
    outT = nc.declare_dram_parameter("outT", [128, 2 * B], f32, isOutput=True).ap()

    eh_np = _consts()
    np_dt = BF16 if use_bf16 else np.float32
    eh_dram = nc.inline_tensor(eh_np.astype(np_dt), name="ehc")

    def mmdt(ap):
        return ap if use_bf16 else ap.bitcast(mybir.dt.float32r)

    # z-mul engine split: GpSimd is ~1.9x slower per element than DVE and
    # DVE also owns the B-block reduces; target ~68/128 z-muls on GpSimd.
    def on_gs(idx):
        return (idx * 17) % 32 < 17

    with tile.TileContext(nc) as tc, ExitStack() as ctx:
        res = ctx.enter_context(tc.tile_pool(name="res", bufs=1))
        eh_sb = res.tile([128, 2, B], dt_big)
        nc.sync.dma_start(out=eh_sb[:, :, :],
                          in_=eh_dram.ap().rearrange("h p b -> p h b"))
        rslots = res.tile([128, nbgB, 2 * B], f32)

        hs_pool = ctx.enter_context(tc.tile_pool(name="hsp", bufs=3))
        dm_pool = ctx.enter_context(tc.tile_pool(name="dmp", bufs=3))
        wf_pool = ctx.enter_context(tc.tile_pool(name="wfp", bufs=3))
        gb_pool = ctx.enter_context(tc.tile_pool(name="gbp", bufs=3))
        z_pool = ctx.enter_context(tc.tile_pool(name="zp", bufs=3))
        psum_h = ctx.enter_context(tc.tile_pool(name="psh", bufs=4, space="PSUM"))
        psum_i = ctx.enter_context(tc.tile_pool(name="psi", bufs=1, space="PSUM"))
        out_pool = ctx.enter_context(tc.tile_pool(name="outp", bufs=1))

        # persistent accumulator [16, (s,o)] = 4 PSUM banks (A-blocks)
        I_ps = psum_i.tile([B, C * no], f32)

        for gc in range(nb):
            dm_t = dm_pool.tile([128, C * no], dt_big, tag="dm")
            nc.sync.dma_start(out=dm_t[:, :], in_=dm[gc])
            wf_t = wf_pool.tile([128, 2 * C * no], dt_big, tag="wf")
            nc.sync.dma_start(out=wf_t[:, :], in_=wf[gc])
            gb_t = gb_pool.tile([128, C * 2 * 128], dt_big, tag="gb")
            nc.sync.dma_start(out=gb_t[:, :], in_=gb[gc])

            gb_v = gb_t.rearrange("p (s h m) -> p s h m", s=C, h=2)
            gb_f = gb_t.rearrange("p (s x) -> p s x", s=C)

            if gc < NA:
                # ---- orientation A: PE does the b-sum (eh matmuls) ----
                wf_v = wf_t.rearrange("p (h s o) -> p h s o", h=2, s=C)
                Z_t = z_pool.tile([128, 2 * C * no], dt_big, tag="z")
                Z_v = Z_t.rearrange("p (h s o) -> p h s o", h=2, s=C)

                for s in range(C):
                    Hp = psum_h.tile([128, 2 * no], f32, tag="hp")
                    so = slice(s * no, (s + 1) * no)
                    for h in range(2):
                        nc.tensor.matmul(Hp[:, h * no:(h + 1) * no],
                                         mmdt(gb_v[:, s, h, :]),
                                         mmdt(dm_t[:, so]),
                                         start=True, stop=True)
                    if on_gs(gc * C + s):
                        Hs = hs_pool.tile([128, 2 * no], dt_big, tag="hs")
                        nc.scalar.copy(Hs[:, :], Hp[:, :])
                        nc.gpsimd.tensor_mul(Z_v[:, :, s, :],
                                             wf_v[:, :, s, :],
                                             Hs.rearrange("p (h o) -> p h o", h=2))
                    else:
                        nc.vector.tensor_mul(Z_v[:, :, s, :],
                                             wf_v[:, :, s, :],
                                             Hp.rearrange("p (h o) -> p h o", h=2))

                Z_h = Z_t.rearrange("p (h x) -> p h x", h=2)
                for h in range(2):
                    for k in range(4):  # 512-col chunks, one PSUM bank each
                        ks = slice(k * 512, (k + 1) * 512)
                        nc.tensor.matmul(I_ps[:, ks],
                                         mmdt(eh_sb[:, h, :]),
                                         mmdt(Z_h[:, h, ks]),
                                         start=(gc == 0 and h == 0),
                                         stop=(gc == NA - 1 and h == 1))
            else:
                # ---- orientation B: DVE does the b-sum (tensor_reduce) ----
                wf_v = wf_t.rearrange("p (s c h b j) -> p s c h b j",
                                      s=C, c=2, h=2, b=HB)
                for s in range(C):
                    HpT = psum_h.tile([128, 2 * no], f32, tag="hp")
                    Hp_v = HpT.rearrange("p (c h b j) -> p c h b j",
                                         c=2, h=2, b=HB)
                    for c2 in range(2):
                        nc.tensor.matmul(HpT[:, c2 * no:(c2 + 1) * no],
                                         mmdt(dm_t[:, s * no + c2 * 128:
                                                   s * no + (c2 + 1) * 128]),
                                         mmdt(gb_f[:, s, :]),
                                         start=True, stop=True)
                    Zt = z_pool.tile([128, 2 * C * no // 8], dt_big, tag="zt")
                    Zt_v = Zt.rearrange("p (c h b j) -> p c h b j", c=2, h=2, b=HB)
                    if on_gs(gc * C + s):
                        Hs = hs_pool.tile([128, 2 * no], dt_big, tag="hs")
                        nc.scalar.copy(Hs[:, :], HpT[:, :])
                        nc.gpsimd.tensor_mul(
                            Zt_v[:, :, :, :, :],
                            wf_v[:, s, :, :, :, :],
                            Hs.rearrange("p (c h b j) -> p c h b j",
                                         c=2, h=2, b=HB))
                    else:
                        nc.vector.tensor_mul(
                            Zt_v[:, :, :, :, :],
                            wf_v[:, s, :, :, :, :],
                            Hp_v[:, :, :, :, :])
                    gi = (gc - NA) * C + s
                    nc.vector.tensor_reduce(rslots[:, gi, :],
                                            Zt_v[:, :, :, :, :],
                                            axis=mybir.AxisListType.X,
                                            op=mybir.AluOpType.add)

        # fold A: [16, (s,o)] viewed as [16, o, s] -> reduce X -> [16, o]
        I_sb = out_pool.tile([B, no], f32)
        nc.vector.tensor_reduce(I_sb[:, :],
                                I_ps.rearrange("b (s o) -> b o s", s=C),
                                axis=mybir.AxisListType.X,
                                op=mybir.AluOpType.add)
        nc.sync.dma_start(out=out, in_=I_sb[:, :])
        # fold B: [128, gi, 32] viewed as [128, 32, gi] -> reduce X
        IT_sb = out_pool.tile([128, 2 * B], f32)
        nc.vector.tensor_reduce(IT_sb[:, :],
                                rslots.rearrange("p g x -> p x g"),
                                axis=mybir.AxisListType.X,
                                op=mybir.AluOpType.add)
        nc.sync.dma_start(out=outT, in_=IT_sb[:, :])

    nc.compile()
    return nc


_CACHE = {}


def kernel(W, Wlong, Wshort, Xd, delaymap, STDP_frac, signs_pre):
    from concourse.bass_utils import run_bass_kernel_spmd

    use_bf16 = os.environ.get("DS_FP32", "0") != "1"
    ins = host_prep(W, Wlong, Wshort, Xd, delaymap, STDP_frac, signs_pre, use_bf16)
    key = ("nc", use_bf16)
    if key not in _CACHE:
        _CACHE[key] = build_nc(use_bf16)
    nc = _CACHE[key]
    r = run_bass_kernel_spmd(nc, ins, list(range(NCORES)))
    outs = []
    for c in range(NCORES):
        oA = r.results[c]["out"].astype(np.float32)          # [16, 256]
        oT = r.results[c]["outT"].astype(np.float32)         # [128, (c2,h,bb)]
        oB = oT.reshape(128, 2, 2, HB).transpose(2, 3, 1, 0).reshape(B, NO)
        outs.append(oA + oB)
    return np.concatenate(outs, axis=1).astype(np.float32)


if __name__ == "__main__":
    pass


# revision 15
# speedup vs baseline: 1.1476x; 1.1476x over previous
"""Trainium2 Bass kernel for nn_DeltaSynapse.

I[b,o] = einsum('beo,dbe,deo,dbe->bo', Weff, Xd, delaymap, Wshort+1)
with Weff[b,e,o] = signs[e,o] * (W[e,o]*(1-frac[e,o]) + Wlong[b,e,o]*frac[e,o])

Identity: I[b,o] = sum_e H2[b,e,o] * Weff[b,e,o],
          H2[b,e,o] = sum_d G[d,b,e] * dm[d,e,o],  G = Xd*(Wshort+1).

Hybrid shard: 2 b-halves x 4 o-quarters (no=512/core). Host computes
Weff; the block-diagonal stationary gb is expanded on device from a
packed gpk (DVE) to keep DMA down. Per e-group g of J=16 e's:
  - gb[(d,j),(bb,j')] = G[d, hb*8+bb, g*16+j]*delta_{j,j'}  (expand)
  - H2 matmul: Hp[(bb,j'), o] = gb.T @ dm[:, g-slice]  (512 cols)
  - Z[(bb,j'), (s,o)] = Hp * Weff-tile   (DVE/GpSimd split)
  - Zred: I_ps[8, (s%4,o)] += eh.T @ Z[:, s, :]  (512-col matmul,
      bank keyed by s%4, accumulated across all blocks)
Final: DVE tensor_reduce folds the 4 bank-chunks -> [8, no] -> out.
"""

import os
import sys
import numpy as np

sys.path.insert(0, "/opt/trn_rl_repo")

import ml_dtypes

BF16 = ml_dtypes.bfloat16

# problem constants
D, B, N = 8, 16, 2048
NCORES = 8
OC = 4            # o-quarters
HBS = 2           # b-halves
NO = N // OC      # per-core o-slice width (512)
J = 16            # e's per group
NG = N // J       # e-groups (128)
HB = B // 2       # b per half (8)
C = 8             # groups per DMA block
NB = NG // C      # DMA blocks (16)


def _consts():
    # eh[p=(bb,j'), bb'] = 1 iff bb' == bb  (bb-major partitions)
    eh = np.zeros((128, HB), dtype=np.float32)
    for bb in range(HB):
        eh[bb * J:(bb + 1) * J, bb] = 1.0
    # dmask[p=(d,j), (s, m=(bb,j'))] = delta_{j, j'}, tiled over s
    p = np.arange(128)
    m = np.arange(128)
    dmask = (p[:, None] % J == m[None, :] % J).astype(np.float32)
    dmask = np.tile(dmask.reshape(128, 1, 128), (1, C, 1)).reshape(128, C * 128)
    return eh, dmask


def host_prep(W, Wlong, Wshort, Xd, delaymap, STDP_frac, signs_pre, use_bf16=True):
    """Host-side prep: Weff fusion, packed G, layout transforms, sharding."""
    dt = BF16 if use_bf16 else np.float32
    W = np.asarray(W, np.float32)
    frac = np.asarray(STDP_frac, np.float32)
    signs = np.where(W > 0, np.sign(np.asarray(signs_pre, np.float32))[:, None],
                     np.float32(0.0))
    A = signs * W * (1.0 - frac)
    SF = signs * frac
    Weff = (A[None] + SF[None] * np.asarray(Wlong, np.float32))  # [B,N,N] f32
    G = (np.asarray(Xd, np.float32) *
         (np.asarray(Wshort, np.float32) + 1.0))  # [D,B,N]

    # dm_r[gc, p=(d,j), (s,o)] = dm[d, (gc*C+s)*J+j, oc*NO+o]
    dmf = np.asarray(delaymap, np.float32)
    dm5 = dmf.reshape(D, NB, C, J, N).transpose(1, 0, 3, 2, 4)  # [NB,D,J,C,N]
    dm_oc = []
    for oc in range(OC):
        sl = slice(oc * NO, (oc + 1) * NO)
        dm_oc.append(np.ascontiguousarray(
            dm5[:, :, :, :, sl].reshape(NB, 128, C * NO)).astype(dt))

    # wf[gc, p=(bb,j'), (s,o)] = Weff[hb*HB+bb, (gc*C+s)*J+j', oc*NO+o]
    wf6 = Weff.reshape(HBS, HB, NB, C, J, N).transpose(0, 2, 1, 4, 3, 5)
    # [hb, NB, HB, J, C, N]

    # gpk[gc, p=(d,j), (s,bb)] = G[d, hb*HB+bb, (gc*C+s)*J+j]
    Gr = G.reshape(D, HBS, HB, NB, C, J)  # [d,hb,bb,gc,s,j]
    gpk_h = Gr.transpose(1, 3, 0, 5, 4, 2)  # [hb, gc, d, j, s, bb]

    ins = []
    for core in range(NCORES):
        hb, oc = core // OC, core % OC
        sl = slice(oc * NO, (oc + 1) * NO)
        ins.append({
            "dm": dm_oc[oc],
            "wf": np.ascontiguousarray(
                wf6[hb, :, :, :, :, sl].reshape(NB, 128, C * NO)).astype(dt),
            "gpk": np.ascontiguousarray(
                gpk_h[hb].reshape(NB, 128, C * HB)).astype(dt),
        })
    return ins


def build_nc(use_bf16=True, n_cores=NCORES, no=NO, ng=NG):
    """Build the SPMD Bass program (same on all cores)."""
    import concourse.bass as bass
    import concourse.bacc as bacc
    import concourse.mybir as mybir
    import concourse.tile as tile
    from contextlib import ExitStack

    dt_big = mybir.dt.bfloat16 if use_bf16 else mybir.dt.float32
    f32 = mybir.dt.float32
    nb = ng // C

    nc = bacc.Bacc("TRN2", target_bir_lowering=False, debug=False,
                   num_devices=n_cores)

    dm = nc.declare_dram_parameter("dm", [nb, 128, C * no], dt_big, isOutput=False).ap()
    wf = nc.declare_dram_parameter("wf", [nb, 128, C * no], dt_big, isOutput=False).ap()
    gpk = nc.declare_dram_parameter("gpk", [nb, 128, C * HB], dt_big, isOutput=False).ap()
    out = nc.declare_dram_parameter("out", [HB, no], f32, isOutput=True).ap()

    eh_np, dmask_np = _consts()
    np_dt = BF16 if use_bf16 else np.float32
    eh_dram = nc.inline_tensor(eh_np.astype(np_dt), name="ehc")
    dmask_dram = nc.inline_tensor(dmask_np.astype(np_dt), name="dmaskc")

    def mmdt(ap):
        return ap if use_bf16 else ap.bitcast(mybir.dt.float32r)

    with tile.TileContext(nc) as tc, ExitStack() as ctx:
        res = ctx.enter_context(tc.tile_pool(name="res", bufs=1))
        eh_sb = res.tile([128, HB], dt_big)
        nc.sync.dma_start(out=eh_sb[:, :], in_=eh_dram.ap())
        dmask_sb = res.tile([128, C * 128], dt_big)
        nc.sync.dma_start(out=dmask_sb[:, :], in_=dmask_dram.ap())

        hs_pool = ctx.enter_context(tc.tile_pool(name="hsp", bufs=3))
        dm_pool = ctx.enter_context(tc.tile_pool(name="dmp", bufs=3))
        wf_pool = ctx.enter_context(tc.tile_pool(name="wfp", bufs=3))
        gp_pool = ctx.enter_context(tc.tile_pool(name="gpp", bufs=3))
        gb_pool = ctx.enter_context(tc.tile_pool(name="gbp", bufs=3))
        z_pool = ctx.enter_context(tc.tile_pool(name="zp", bufs=3))
        psum_h = ctx.enter_context(tc.tile_pool(name="psh", bufs=4, space="PSUM"))
        psum_i = ctx.enter_context(tc.tile_pool(name="psi", bufs=1, space="PSUM"))
        out_pool = ctx.enter_context(tc.tile_pool(name="outp", bufs=1))

        # persistent accumulator [8, (s%4,o)] = 4 PSUM banks
        I_ps = psum_i.tile([HB, 4 * no], f32)

        # z-mul engine split: GpSimd ~1.9x slower than DVE; DVE also does
        # the gb expansion (1 op/block).
        GS_SLOTS = (2, 5, 7)

        for gc in range(nb):
            dm_t = dm_pool.tile([128, C * no], dt_big, tag="dm")
            nc.sync.dma_start(out=dm_t[:, :], in_=dm[gc])
            wf_t = wf_pool.tile([128, C * no], dt_big, tag="wf")
            nc.sync.dma_start(out=wf_t[:, :], in_=wf[gc])
            gp_t = gp_pool.tile([128, C * HB], dt_big, tag="gp")
            nc.sync.dma_start(out=gp_t[:, :], in_=gpk[gc])

            # expand gb[p=(d,j), (s, m=(bb,j'))] = gpk[p,(s,bb)]*dmask[p,j']
            gb_t = gb_pool.tile([128, C * 128], dt_big, tag="gb")
            nc.vector.tensor_mul(
                gb_t.rearrange("p (s b j) -> p s b j", s=C, b=HB),
                gp_t.rearrange("p (s b) -> p s b", s=C).unsqueeze(3)
                    .broadcast_to((128, C, HB, J)),
                dmask_sb.rearrange("p (s b j) -> p s b j", s=C, b=HB))

            gb_v = gb_t.rearrange("p (s m) -> p s m", s=C)
            wf_v = wf_t.rearrange("p (s o) -> p s o", s=C)
            Z_t = z_pool.tile([128, C * no], dt_big, tag="z")
            Z_v = Z_t.rearrange("p (s o) -> p s o", s=C)

            for s in range(C):
                Hp = psum_h.tile([128, no], f32, tag="hp")
                so = slice(s * no, (s + 1) * no)
                nc.tensor.matmul(Hp[:, :],
                                 mmdt(gb_v[:, s, :]),
                                 mmdt(dm_t[:, so]),
                                 start=True, stop=True)
                if s in GS_SLOTS:
                    # GpSimd cannot read PSUM: ACT evacuates to SBUF bf16
                    Hs = hs_pool.tile([128, no], dt_big, tag="hs")
                    nc.scalar.copy(Hs[:, :], Hp[:, :])
                    nc.gpsimd.tensor_mul(Z_v[:, s, :], wf_v[:, s, :], Hs[:, :])
                else:
                    nc.vector.tensor_mul(Z_v[:, s, :], wf_v[:, s, :], Hp[:, :])

                nc.tensor.matmul(I_ps[:, (s % 4) * no:(s % 4 + 1) * no],
                                 mmdt(eh_sb[:, :]),
                                 mmdt(Z_v[:, s, :]),
                                 start=(gc == 0 and s < 4),
                                 stop=(gc == nb - 1 and s >= 4))

        # fold bank-chunks: [8, (k,o)] viewed as [8, o, k] -> reduce X
        I_sb = out_pool.tile([HB, no], f32)
        nc.vector.tensor_reduce(I_sb[:, :],
                                I_ps.rearrange("b (k o) -> b o k", k=4),
                                axis=mybir.AxisListType.X,
                                op=mybir.AluOpType.add)
        nc.sync.dma_start(out=out, in_=I_sb[:, :])

    nc.compile()
    return nc


_CACHE = {}


def kernel(W, Wlong, Wshort, Xd, delaymap, STDP_frac, signs_pre):
    from concourse.bass_utils import run_bass_kernel_spmd

    use_bf16 = os.environ.get("DS_FP32", "0") != "1"
    ins = host_prep(W, Wlong, Wshort, Xd, delaymap, STDP_frac, signs_pre, use_bf16)
    key = ("nc", use_bf16)
    if key not in _CACHE:
        _CACHE[key] = build_nc(use_bf16)
    nc = _CACHE[key]
    r = run_bass_kernel_spmd(nc, ins, list(range(NCORES)))
    out_full = np.zeros((B, N), np.float32)
    for core in range(NCORES):
        hb, oc = core // OC, core % OC
        out_full[hb * HB:(hb + 1) * HB, oc * NO:(oc + 1) * NO] = \
            r.results[core]["out"].astype(np.float32)
    return out_full


if __name__ == "__main__":
    pass
